# revision 30
# baseline (speedup 1.0000x reference)
"""Bass/Tile TRN2 kernel for nn_MultiHeadSeqAttention_82789789597729.

Math: the reference's softmax / positional scores are dead code -- its output
is exactly  out = concat_h(q_h @ k_h^T @ v_h) @ Wo^T  with no nonlinearity.
By associativity  q (k^T v)  replaces the [M,M] score matrix with a [D,D]
one, collapsing ~69 GFLOP to ~26 GFLOP.

Sharding: tensor-parallel over heads (4 heads / core) x data-parallel over
batch (B=2) -> 8 cores. Each core computes a full-M partial output for its
head group; the host sums the 4 partials per batch (row-parallel unshard).

Schedule: tuned so the PE never idles (its p-state ramps to 2.4 GHz only
after ~3us of continuous work). Stage A (k^T v) is pair-stacked into
[128,128] matmuls interleaved with the kv projection, accumulating in a
persistent PSUM tile. DMA uses few large kicks ordered by consumption;
output kicks ride the otherwise-idle sync engine; PSUM->SBUF casts are
spread across vector/gpsimd/scalar.
"""

import numpy as np
import ml_dtypes

import concourse.bass as bass
import concourse.mybir as mybir
import concourse.tile as tile
from concourse.bass_utils import run_bass_kernel_spmd
from concourse.vector_clock import ScopedClock
import bass_rust

B, M, H, K, D = 2, 2048, 1024, 16, 64
N_CORES = 8
HPC = 4           # heads per core
CC = HPC * D      # 256 local feature columns per core
P = 128

MM_DT = "f16"


# --- workaround: this walrus rejects multi-wait Drain instructions, so split
# --- the TileContext exit drain into one single-wait drain per proc.
def _split_drain_and_barrier(self, tick_clock, wait_clock):
    n_procs = len(list(tick_clock.global_clock))
    for p, t in enumerate(tick_clock.global_clock):
        if t <= 0:
            continue
        single = bass_rust.VectorClock(
            [t if i == p else 0 for i in range(n_procs)]
        )
        d = self.nc.sync.drain()
        wait_clock.add_sem_waits(d.ins, ScopedClock({None: single}))
    self.nc.all_engine_barrier()
    popped = self.nc._tile_sem_poison_stack.pop()
    assert popped is self._sem_poison
    self.nc.clear_and_free_semaphores(list(self.sems.allocated().values()))
    self.nc.all_engine_barrier()


# --- workaround: the same walrus caps sync waits at 1 per instruction
# --- (2 for EventSemaphore). Tile's wait-assignment can attach more; hoist
# --- the extras onto single-wait nop carriers emitted just before.
_ORIG_COMMIT_AND_LOWER = tile.TileContext._commit_and_lower


def _wait_split_commit_and_lower(self, inst, original_block, old_bb_map,
                                 bb_to_exit_bb):
    si = inst.sync_info
    cap = 2 if isinstance(inst, mybir.InstEventSemaphore) else 1
    ow = list(si.on_wait) if si is not None and si.on_wait else []
    if len(ow) > cap and inst.is_executable():
        for w in ow[:-cap]:
            carrier = self.nc.engines[inst.engine].nop(nofuse=True)
            carrier.ins.sync_info = bass_rust.SyncInfo(
                on_wait=[w], on_update=[]
            )
        inst.sync_info = bass_rust.SyncInfo(
            on_wait=ow[-cap:], on_update=list(si.on_update or [])
        )
    return _ORIG_COMMIT_AND_LOWER(
        self, inst, original_block, old_bb_map, bb_to_exit_bb
    )


if not getattr(tile.TileContext, "_split_drain_patched", False):
    tile.TileContext._drain_and_barrier = _split_drain_and_barrier
    tile.TileContext._commit_and_lower = _wait_split_commit_and_lower
    tile.TileContext._split_drain_patched = True


def _build_nc():
    if MM_DT == "bf16":
        io_dt = mybir.dt.bfloat16
    elif MM_DT == "f16":
        io_dt = mybir.dt.float16
    elif MM_DT == "f32r":
        io_dt = mybir.dt.float32r
    else:
        io_dt = mybir.dt.float32
    f32 = mybir.dt.float32

    nc = bass.Bass()
    IT = H // P           # 8 contraction tiles over feature dim
    LT = M // P           # 16 tiles over sequence dim
    MC = M // 512         # 4 moving chunks over sequence dim
    DT = CC // P          # 2 partition tiles over local feature cols
    JC = H // 512         # 2 chunks over output feature dim
    NCH = 8               # hc DMA chunks (2 seq-tiles each)

    # host pre-arranges every input into its SBUF layout so each DMA kick
    # is one long descriptor per partition (per-ring DMA bandwidth scales
    # with descriptor size). hc is split: four small 128-col chunks for a
    # fast pipeline start, then three 512-col chunks with 8KB descriptors.
    hC = nc.dram_tensor("hC", [P, MC, IT, 512], io_dt, kind="ExternalInput")
    hcA = nc.dram_tensor("hcA", [P, 4, IT, 128], io_dt, kind="ExternalInput")
    hcB = nc.dram_tensor("hcB", [P, 3, IT, 512], io_dt, kind="ExternalInput")
    wq_d = nc.dram_tensor("wq", [P, IT, CC], io_dt, kind="ExternalInput")
    wkv_d = nc.dram_tensor("wkv", [P, IT, 2 * CC], io_dt,
                           kind="ExternalInput")
    wo_d = nc.dram_tensor("wo", [D, HPC, H], io_dt, kind="ExternalInput")
    out_dt = mybir.dt.float16 if MM_DT == "f16" else f32
    outp = nc.dram_tensor("out", [M, H], out_dt, kind="ExternalOutput")

    with tile.TileContext(nc) as tc:
        with (
            tc.tile_pool(name="wp", bufs=1) as wp,
            tc.tile_pool(name="big", bufs=1) as big,
            tc.tile_pool(name="op", bufs=4) as op,
            tc.tile_pool(name="ps", bufs=2, space="PSUM") as ps,
            tc.tile_pool(name="po", bufs=4, space="PSUM") as po,
            tc.tile_pool(name="pa", bufs=2, space="PSUM") as pa,
        ):
            wkv_sb = wp.tile([P, IT, 2 * CC], io_dt, tag="wkv")
            wq_sb = wp.tile([P, IT, CC], io_dt, tag="wq")
            wo_sb = wp.tile([D, HPC, H], io_dt, tag="wo")
            hcA_sb = wp.tile([P, 4, IT, 128], io_dt, tag="hca")
            hcB_sb = wp.tile([P, 3, IT, 512], io_dt, tag="hcb")
            h_sb = wp.tile([P, MC, IT, 512], io_dt, tag="h")

            # head-critical loads fan out across four rings (per-ring DMA
            # bandwidth is well below the HBM cap); weights finish on sync
            nc.sync.dma_start(out=wkv_sb[:, 0:4, :], in_=wkv_d[:, 0:4, :])
            nc.scalar.dma_start(out=wkv_sb[:, 4:8, :], in_=wkv_d[:, 4:8, :])
            nc.gpsimd.dma_start(out=hcA_sb[:, 0], in_=hcA[:, 0])
            nc.gpsimd.dma_start(out=hcA_sb[:, 1], in_=hcA[:, 1])
            nc.scalar.dma_start(out=hcA_sb[:, 2], in_=hcA[:, 2])
            nc.gpsimd.dma_start(out=hcA_sb[:, 3], in_=hcA[:, 3])
            nc.scalar.dma_start(out=hcB_sb[:, 0], in_=hcB[:, 0])
            nc.gpsimd.dma_start(out=hcB_sb[:, 1], in_=hcB[:, 1])
            nc.scalar.dma_start(out=hcB_sb[:, 2], in_=hcB[:, 2])
            nc.sync.dma_start(out=wq_sb[:], in_=wq_d[:])
            nc.sync.dma_start(out=wo_sb[:], in_=wo_d[:])
            engs = [nc.gpsimd, nc.scalar]
            for mc in range(MC):
                engs[mc % 2].dma_start(out=h_sb[:, mc], in_=hC[:, mc])

            def hc_tile(it, lt):
                if lt < 4:
                    return hcA_sb[:, lt, it, :]
                ch, sub = (lt - 4) // 4, (lt - 4) % 4
                return hcB_sb[:, ch, it, sub * P:(sub + 1) * P]

            # persistent intermediates
            kv_sb = big.tile([P, LT, 2 * CC], io_dt, tag="kv")
            q_sb = big.tile([P, DT, M], io_dt, tag="q")
            at_sb = big.tile([D, HPC, D], io_dt, tag="at")
            c_sb = big.tile([P, DT, H], io_dt, tag="c")

            # --- P1 (kv projection) with stage A (pair-stacked v^T k)
            # interleaved one seq-tile behind (so the PE never waits on the
            # kv cast); A accumulates in persistent PSUM tiles, one full
            # bank per head-pair (start=True clears the whole bank).
            at_ps = [
                pa.tile([P, 512], f32, tag="pa", name=f"at_ps{pp}")
                for pp in range(DT)
            ]

            def a_mms(lt):
                for pp in range(DT):
                    nc.tensor.matmul(
                        at_ps[pp][:, 0:P],
                        kv_sb[:, lt, CC + pp * P:CC + (pp + 1) * P],
                        kv_sb[:, lt, pp * P:(pp + 1) * P],
                        start=(lt == 0), stop=(lt == LT - 1),
                        skip_group_check=True,
                    )

            for lt in range(LT):
                acc = ps.tile([P, 2 * CC], f32, tag="ps")
                for it in range(IT):
                    nc.tensor.matmul(
                        acc[:],
                        hc_tile(it, lt),
                        wkv_sb[:, it, :],
                        start=(it == 0), stop=(it == IT - 1),
                    )
                nc.vector.tensor_copy(kv_sb[:, lt, :], acc[:])
                if lt > 0:
                    a_mms(lt - 1)
            a_mms(LT - 1)
            # extract per-head diagonal blocks into baseline at layout
            for hh in range(HPC):
                pp, i = hh // 2, hh % 2
                nc.vector.tensor_copy(
                    at_sb[:, hh, :],
                    at_ps[pp][i * D:(i + 1) * D, i * D:(i + 1) * D],
                )

            # --- stage C: rows of (A_h Wo_h^T) [du, j]
            for hh in range(HPC):
                prow = (hh % 2) * D
                for jc in range(JC):
                    cacc = ps.tile([D, 512], f32, tag="ps")
                    nc.tensor.matmul(
                        cacc[:],
                        at_sb[:, hh, :],
                        wo_sb[:, hh, jc * 512:(jc + 1) * 512],
                        start=True, stop=True,
                    )
                    nc.vector.tensor_copy(
                        c_sb[prow:prow + D, hh // 2, jc * 512:(jc + 1) * 512],
                        cacc[:],
                    )

            # --- P2 (q projection) + O (q @ C) interleaved per 512-col
            # chunk of m; casts rotate across vector/gpsimd/scalar and
            # output kicks ride the sync engine.
            # gpsimd cannot access PSUM; rotate PSUM->SBUF casts over
            # vector and scalar only
            cast_engs = [
                lambda o, i: nc.vector.tensor_copy(o, i),
                lambda o, i: nc.scalar.copy(o, i),
            ]
            n_cast = 0
            for mc in range(MC):
                for dt_i in range(DT):
                    qacc = ps.tile([P, 512], f32, tag="ps")
                    for it in range(IT):
                        nc.tensor.matmul(
                            qacc[:],
                            wq_sb[:, it, dt_i * P:(dt_i + 1) * P],
                            h_sb[:, mc, it, :],
                            start=(it == 0), stop=(it == IT - 1),
                        )
                    nc.vector.tensor_copy(
                        q_sb[:, dt_i, mc * 512:(mc + 1) * 512], qacc[:]
                    )
                for mt in range(mc * 4, (mc + 1) * 4):
                    o_t = op.tile([P, H], out_dt, tag="o")
                    # dt-outer: the first two matmuls need only the dt=0
                    # q cast, hiding the dt=1 cast latency
                    oaccs = [po.tile([P, 512], f32, tag="po",
                                     name=f"oacc{mt}_{jc}")
                             for jc in range(JC)]
                    for dt_i in range(DT):
                        for jc in range(JC):
                            nc.tensor.matmul(
                                oaccs[jc][:],
                                q_sb[:, dt_i, mt * P:(mt + 1) * P],
                                c_sb[:, dt_i, jc * 512:(jc + 1) * 512],
                                start=(dt_i == 0), stop=(dt_i == DT - 1),
                                skip_group_check=True,
                            )
                    for jc in range(JC):
                        cast_engs[n_cast % 2](
                            o_t[:, jc * 512:(jc + 1) * 512], oaccs[jc][:]
                        )
                        n_cast += 1
                    # output rides two rings: per-ring DMA throughput is
                    # well under the HBM cap
                    (nc.sync if mt % 2 == 0 else nc.gpsimd).dma_start(
                        out=outp[mt * P:(mt + 1) * P, :], in_=o_t[:]
                    )

    return nc


_NC_CACHE = {}


def _get_nc():
    if "nc" not in _NC_CACHE:
        _NC_CACHE["nc"] = _build_nc()
    return _NC_CACHE["nc"]


def _cast(a):
    a = np.ascontiguousarray(a)
    if MM_DT == "bf16":
        return a.astype(ml_dtypes.bfloat16)
    if MM_DT == "f16":
        return a.astype(np.float16)
    return a.astype(np.float32)


def make_in_maps(h, h_cache, Wq, Wk, Wv, Wo):
    # pre-arrange every input into its SBUF layout (partition-major with
    # contiguous per-partition chunks) so device DMA descriptors are long
    IT, MC, NCH = H // 128, 4, 8

    def t_chunks(xT, c0, c1, w):
        # [H, M] cols [c0:c1] -> [128 p, nch, IT it, w]
        nch = (c1 - c0) // w
        return _cast(
            np.ascontiguousarray(
                xT[:, c0:c1].reshape(IT, 128, nch, w).transpose(1, 2, 0, 3))
        )

    in_maps = []
    for c in range(N_CORES):
        b, g = divmod(c, 4)
        cols = slice(g * CC, (g + 1) * CC)
        wkvT = np.concatenate([Wk[cols, :].T, Wv[cols, :].T], axis=1)
        in_maps.append({
            "hC": t_chunks(h[b].T, 0, M, 512),
            "hcA": t_chunks(h_cache[b].T, 0, 512, 128),
            "hcB": t_chunks(h_cache[b].T, 512, M, 512),
            "wq": _cast(np.ascontiguousarray(
                Wq[cols, :].T.reshape(IT, 128, CC).transpose(1, 0, 2))),
            "wkv": _cast(np.ascontiguousarray(
                wkvT.reshape(IT, 128, 2 * CC).transpose(1, 0, 2))),
            "wo": _cast(np.ascontiguousarray(
                Wo[:, cols].T.reshape(HPC, D, H).transpose(1, 0, 2))),
        })
    return in_maps


def kernel(h, h_cache, key_pe, Wq, Wk, Wv, Wo, _bass_results=None):
    h = np.asarray(h)
    h_cache = np.asarray(h_cache)
    Wq, Wk, Wv, Wo = (np.asarray(a) for a in (Wq, Wk, Wv, Wo))
    nc = _get_nc()
    in_maps = make_in_maps(h, h_cache, Wq, Wk, Wv, Wo)
    res = run_bass_kernel_spmd(nc, in_maps, list(range(N_CORES)))
    if _bass_results is not None:
        _bass_results.append(res)
    out = np.zeros((B, M, H), np.float32)
    for c in range(N_CORES):
        out[c // 4] += res.results[c]["out"].astype(np.float32)
    return out


# revision 35
# speedup vs baseline: 1.1199x; 1.1199x over previous
"""Bass/Tile TRN2 kernel for nn_MultiHeadSeqAttention_82789789597729.

Math: the reference's softmax / positional scores are dead code -- its output
is exactly  out = concat_h(q_h @ k_h^T @ v_h) @ Wo^T  with no nonlinearity.
By associativity  q (k^T v)  replaces the [M,M] score matrix with a [D,D]
one, collapsing ~69 GFLOP to ~26 GFLOP.

Sharding: tensor-parallel over heads (4 heads / core) x data-parallel over
batch (B=2) -> 8 cores. Each core computes a full-M partial output for its
head group; the host sums the 4 partials per batch (row-parallel unshard).

Schedule: tuned so the PE never idles (its p-state ramps to 2.4 GHz only
after ~3us of continuous work). Stage A (k^T v) is pair-stacked into
[128,128] matmuls interleaved with the kv projection, accumulating in a
persistent PSUM tile. DMA uses few large kicks ordered by consumption;
output kicks ride the otherwise-idle sync engine; PSUM->SBUF casts are
spread across vector/gpsimd/scalar.
"""

import numpy as np
import ml_dtypes

import concourse.bass as bass
import concourse.mybir as mybir
import concourse.tile as tile
from concourse.bass_utils import run_bass_kernel_spmd
from concourse.vector_clock import ScopedClock
import bass_rust

B, M, H, K, D = 2, 2048, 1024, 16, 64
N_CORES = 8
HPC = 4           # heads per core
CC = HPC * D      # 256 local feature columns per core
P = 128

MM_DT = "f16"


# --- workaround: this walrus rejects multi-wait Drain instructions, so split
# --- the TileContext exit drain into one single-wait drain per proc.
def _split_drain_and_barrier(self, tick_clock, wait_clock):
    n_procs = len(list(tick_clock.global_clock))
    for p, t in enumerate(tick_clock.global_clock):
        if t <= 0:
            continue
        single = bass_rust.VectorClock(
            [t if i == p else 0 for i in range(n_procs)]
        )
        d = self.nc.sync.drain()
        wait_clock.add_sem_waits(d.ins, ScopedClock({None: single}))
    self.nc.all_engine_barrier()
    popped = self.nc._tile_sem_poison_stack.pop()
    assert popped is self._sem_poison
    self.nc.clear_and_free_semaphores(list(self.sems.allocated().values()))
    self.nc.all_engine_barrier()


# --- workaround: the same walrus caps sync waits at 1 per instruction
# --- (2 for EventSemaphore). Tile's wait-assignment can attach more; hoist
# --- the extras onto single-wait nop carriers emitted just before.
_ORIG_COMMIT_AND_LOWER = tile.TileContext._commit_and_lower


def _wait_split_commit_and_lower(self, inst, original_block, old_bb_map,
                                 bb_to_exit_bb):
    si = inst.sync_info
    cap = 2 if isinstance(inst, mybir.InstEventSemaphore) else 1
    ow = list(si.on_wait) if si is not None and si.on_wait else []
    if len(ow) > cap and inst.is_executable():
        for w in ow[:-cap]:
            carrier = self.nc.engines[inst.engine].nop(nofuse=True)
            carrier.ins.sync_info = bass_rust.SyncInfo(
                on_wait=[w], on_update=[]
            )
        inst.sync_info = bass_rust.SyncInfo(
            on_wait=ow[-cap:], on_update=list(si.on_update or [])
        )
    return _ORIG_COMMIT_AND_LOWER(
        self, inst, original_block, old_bb_map, bb_to_exit_bb
    )


if not getattr(tile.TileContext, "_split_drain_patched", False):
    tile.TileContext._drain_and_barrier = _split_drain_and_barrier
    tile.TileContext._commit_and_lower = _wait_split_commit_and_lower
    tile.TileContext._split_drain_patched = True


def _build_nc():
    if MM_DT == "bf16":
        io_dt = mybir.dt.bfloat16
    elif MM_DT == "f16":
        io_dt = mybir.dt.float16
    elif MM_DT == "f32r":
        io_dt = mybir.dt.float32r
    else:
        io_dt = mybir.dt.float32
    f32 = mybir.dt.float32

    nc = bass.Bass()
    IT = H // P           # 8 contraction tiles over feature dim
    LT = M // P           # 16 tiles over sequence dim
    MC = M // 512         # 4 moving chunks over sequence dim
    DT = CC // P          # 2 partition tiles over local feature cols
    JC = H // 512         # 2 chunks over output feature dim
    NCH = 8               # hc DMA chunks (2 seq-tiles each)

    # host pre-arranges every input into its SBUF layout so each DMA kick
    # is one long descriptor per partition (per-ring DMA bandwidth scales
    # with descriptor size). hc is split: four small 128-col chunks for a
    # fast pipeline start, then three 512-col chunks with 8KB descriptors.
    hC = nc.dram_tensor("hC", [P, MC, IT, 512], io_dt, kind="ExternalInput")
    hcA = nc.dram_tensor("hcA", [P, 8, IT, 128], io_dt, kind="ExternalInput")
    hcB = nc.dram_tensor("hcB", [P, 2, IT, 512], io_dt, kind="ExternalInput")
    wq_d = nc.dram_tensor("wq", [P, IT, CC], io_dt, kind="ExternalInput")
    wkv_d = nc.dram_tensor("wkv", [P, IT, 2 * CC], io_dt,
                           kind="ExternalInput")
    wo_d = nc.dram_tensor("wo", [D, HPC, H], io_dt, kind="ExternalInput")
    out_dt = mybir.dt.float16 if MM_DT == "f16" else f32
    outp = nc.dram_tensor("out", [M, H], out_dt, kind="ExternalOutput")

    with tile.TileContext(nc) as tc:
        with (
            tc.tile_pool(name="wp", bufs=1) as wp,
            tc.tile_pool(name="big", bufs=1) as big,
            tc.tile_pool(name="op", bufs=4) as op,
            tc.tile_pool(name="ps", bufs=2, space="PSUM") as ps,
            tc.tile_pool(name="po", bufs=4, space="PSUM") as po,
            tc.tile_pool(name="pa", bufs=2, space="PSUM") as pa,
        ):
            wkv_sb = wp.tile([P, IT, 2 * CC], io_dt, tag="wkv")
            wq_sb = wp.tile([P, IT, CC], io_dt, tag="wq")
            wo_sb = wp.tile([D, HPC, H], io_dt, tag="wo")
            hcA_sb = wp.tile([P, 8, IT, 128], io_dt, tag="hca")
            hcB_sb = wp.tile([P, 2, IT, 512], io_dt, tag="hcb")
            h_sb = wp.tile([P, MC, IT, 512], io_dt, tag="h")

            # per-ring DMA bandwidth is well below the HBM cap, so the kick
            # schedule balances each ring's cumulative bytes against the
            # consumption time of each chunk (P1 eats hc chunks in order)
            nc.sync.dma_start(out=wkv_sb[:, 0:4, :], in_=wkv_d[:, 0:4, :])
            nc.gpsimd.dma_start(out=wkv_sb[:, 4:8, :], in_=wkv_d[:, 4:8, :])
            nc.scalar.dma_start(out=hcA_sb[:, 0], in_=hcA[:, 0])
            nc.gpsimd.dma_start(out=hcA_sb[:, 1], in_=hcA[:, 1])
            nc.scalar.dma_start(out=hcA_sb[:, 2], in_=hcA[:, 2])
            nc.sync.dma_start(out=hcA_sb[:, 3], in_=hcA[:, 3])
            nc.gpsimd.dma_start(out=hcA_sb[:, 4], in_=hcA[:, 4])
            nc.scalar.dma_start(out=hcA_sb[:, 5], in_=hcA[:, 5])
            nc.sync.dma_start(out=hcA_sb[:, 6], in_=hcA[:, 6])
            nc.gpsimd.dma_start(out=hcA_sb[:, 7], in_=hcA[:, 7])
            nc.scalar.dma_start(out=hcB_sb[:, 0], in_=hcB[:, 0])
            nc.gpsimd.dma_start(out=hcB_sb[:, 1], in_=hcB[:, 1])
            nc.sync.dma_start(out=wq_sb[:], in_=wq_d[:])
            nc.sync.dma_start(out=wo_sb[:], in_=wo_d[:])
            engs = [nc.scalar, nc.gpsimd]
            for mc in range(MC):
                engs[mc % 2].dma_start(out=h_sb[:, mc], in_=hC[:, mc])

            def hc_tile(it, lt):
                if lt < 8:
                    return hcA_sb[:, lt, it, :]
                ch, sub = (lt - 8) // 4, (lt - 8) % 4
                return hcB_sb[:, ch, it, sub * P:(sub + 1) * P]

            # persistent intermediates
            kv_sb = big.tile([P, LT, 2 * CC], io_dt, tag="kv")
            q_sb = big.tile([P, DT, M], io_dt, tag="q")
            at_sb = big.tile([D, HPC, D], io_dt, tag="at")
            c_sb = big.tile([P, DT, H], io_dt, tag="c")

            # --- P1 (kv projection) with stage A (pair-stacked v^T k)
            # interleaved one seq-tile behind (so the PE never waits on the
            # kv cast); A accumulates in persistent PSUM tiles, one full
            # bank per head-pair (start=True clears the whole bank).
            at_ps = [
                pa.tile([P, 512], f32, tag="pa", name=f"at_ps{pp}")
                for pp in range(DT)
            ]

            def a_mms(lt):
                for pp in range(DT):
                    nc.tensor.matmul(
                        at_ps[pp][:, 0:P],
                        kv_sb[:, lt, CC + pp * P:CC + (pp + 1) * P],
                        kv_sb[:, lt, pp * P:(pp + 1) * P],
                        start=(lt == 0), stop=(lt == LT - 1),
                        skip_group_check=True,
                    )

            for lt in range(LT):
                acc = ps.tile([P, 2 * CC], f32, tag="ps")
                for it in range(IT):
                    nc.tensor.matmul(
                        acc[:],
                        hc_tile(it, lt),
                        wkv_sb[:, it, :],
                        start=(it == 0), stop=(it == IT - 1),
                    )
                nc.vector.tensor_copy(kv_sb[:, lt, :], acc[:])
                if lt > 0:
                    a_mms(lt - 1)
            a_mms(LT - 1)
            # extract per-head diagonal blocks into baseline at layout
            for hh in range(HPC):
                pp, i = hh // 2, hh % 2
                nc.vector.tensor_copy(
                    at_sb[:, hh, :],
                    at_ps[pp][i * D:(i + 1) * D, i * D:(i + 1) * D],
                )

            def c_stage():
                # rows of (A_h Wo_h^T) [du, j]
                for hh in range(HPC):
                    prow = (hh % 2) * D
                    for jc in range(JC):
                        cacc = ps.tile([D, 512], f32, tag="ps")
                        nc.tensor.matmul(
                            cacc[:],
                            at_sb[:, hh, :],
                            wo_sb[:, hh, jc * 512:(jc + 1) * 512],
                            start=True, stop=True,
                        )
                        nc.vector.tensor_copy(
                            c_sb[prow:prow + D, hh // 2,
                                 jc * 512:(jc + 1) * 512],
                            cacc[:],
                        )

            # --- P2 (q projection) + O (q @ C) interleaved per 512-col
            # chunk of m; casts rotate across vector/gpsimd/scalar and
            # output kicks ride the sync engine.
            # gpsimd cannot access PSUM; rotate PSUM->SBUF casts over
            # vector and scalar only
            cast_engs = [
                lambda o, i: nc.vector.tensor_copy(o, i),
                lambda o, i: nc.scalar.copy(o, i),
            ]
            n_cast = 0
            def p2_stage(mc):
                for dt_i in range(DT):
                    qacc = ps.tile([P, 512], f32, tag="ps")
                    for it in range(IT):
                        nc.tensor.matmul(
                            qacc[:],
                            wq_sb[:, it, dt_i * P:(dt_i + 1) * P],
                            h_sb[:, mc, it, :],
                            start=(it == 0), stop=(it == IT - 1),
                        )
                    nc.vector.tensor_copy(
                        q_sb[:, dt_i, mc * 512:(mc + 1) * 512], qacc[:]
                    )

            # P2(0) runs between A and C so C's dependency chain (A stop ->
            # extraction cast -> C matmul) resolves off the critical path
            p2_stage(0)
            c_stage()
            for mc in range(MC):
                if mc > 0:
                    p2_stage(mc)
                for mt in range(mc * 4, (mc + 1) * 4):
                    o_t = op.tile([P, H], out_dt, tag="o")
                    # dt-outer: the first two matmuls need only the dt=0
                    # q cast, hiding the dt=1 cast latency
                    oaccs = [po.tile([P, 512], f32, tag="po",
                                     name=f"oacc{mt}_{jc}")
                             for jc in range(JC)]
                    for dt_i in range(DT):
                        for jc in range(JC):
                            nc.tensor.matmul(
                                oaccs[jc][:],
                                q_sb[:, dt_i, mt * P:(mt + 1) * P],
                                c_sb[:, dt_i, jc * 512:(jc + 1) * 512],
                                start=(dt_i == 0), stop=(dt_i == DT - 1),
                                skip_group_check=True,
                            )
                    for jc in range(JC):
                        cast_engs[n_cast % 2](
                            o_t[:, jc * 512:(jc + 1) * 512], oaccs[jc][:]
                        )
                        n_cast += 1
                    # output rides two rings: per-ring DMA throughput is
                    # well under the HBM cap
                    (nc.sync if mt % 2 == 0 else nc.gpsimd).dma_start(
                        out=outp[mt * P:(mt + 1) * P, :], in_=o_t[:]
                    )

    return nc


_NC_CACHE = {}


def _get_nc():
    if "nc" not in _NC_CACHE:
        _NC_CACHE["nc"] = _build_nc()
    return _NC_CACHE["nc"]


def _cast(a):
    a = np.ascontiguousarray(a)
    if MM_DT == "bf16":
        return a.astype(ml_dtypes.bfloat16)
    if MM_DT == "f16":
        return a.astype(np.float16)
    return a.astype(np.float32)


def make_in_maps(h, h_cache, Wq, Wk, Wv, Wo):
    # pre-arrange every input into its SBUF layout (partition-major with
    # contiguous per-partition chunks) so device DMA descriptors are long
    IT, MC, NCH = H // 128, 4, 8

    def t_chunks(xT, c0, c1, w):
        # [H, M] cols [c0:c1] -> [128 p, nch, IT it, w]
        nch = (c1 - c0) // w
        return _cast(
            np.ascontiguousarray(
                xT[:, c0:c1].reshape(IT, 128, nch, w).transpose(1, 2, 0, 3))
        )

    in_maps = []
    for c in range(N_CORES):
        b, g = divmod(c, 4)
        cols = slice(g * CC, (g + 1) * CC)
        wkvT = np.concatenate([Wk[cols, :].T, Wv[cols, :].T], axis=1)
        in_maps.append({
            "hC": t_chunks(h[b].T, 0, M, 512),
            "hcA": t_chunks(h_cache[b].T, 0, 1024, 128),
            "hcB": t_chunks(h_cache[b].T, 1024, M, 512),
            "wq": _cast(np.ascontiguousarray(
                Wq[cols, :].T.reshape(IT, 128, CC).transpose(1, 0, 2))),
            "wkv": _cast(np.ascontiguousarray(
                wkvT.reshape(IT, 128, 2 * CC).transpose(1, 0, 2))),
            "wo": _cast(np.ascontiguousarray(
                Wo[:, cols].T.reshape(HPC, D, H).transpose(1, 0, 2))),
        })
    return in_maps


def kernel(h, h_cache, key_pe, Wq, Wk, Wv, Wo, _bass_results=None):
    h = np.asarray(h)
    h_cache = np.asarray(h_cache)
    Wq, Wk, Wv, Wo = (np.asarray(a) for a in (Wq, Wk, Wv, Wo))
    nc = _get_nc()
    in_maps = make_in_maps(h, h_cache, Wq, Wk, Wv, Wo)
    res = run_bass_kernel_spmd(nc, in_maps, list(range(N_CORES)))
    if _bass_results is not None:
        _bass_results.append(res)
    out = np.zeros((B, M, H), np.float32)
    for c in range(N_CORES):
        out[c // 4] += res.results[c]["out"].astype(np.float32)
    return out


# revision 37
# speedup vs baseline: 1.1327x; 1.0115x over previous
"""Bass/Tile TRN2 kernel for nn_MultiHeadSeqAttention_82789789597729.

Math: the reference's softmax / positional scores are dead code -- its output
is exactly  out = concat_h(q_h @ k_h^T @ v_h) @ Wo^T  with no nonlinearity.
By associativity  q (k^T v)  replaces the [M,M] score matrix with a [D,D]
one, collapsing ~69 GFLOP to ~26 GFLOP.

Sharding: tensor-parallel over heads (4 heads / core) x data-parallel over
batch (B=2) -> 8 cores. Each core computes a full-M partial output for its
head group; the host sums the 4 partials per batch (row-parallel unshard).

Schedule: tuned so the PE never idles (its p-state ramps to 2.4 GHz only
after ~3us of continuous work). Stage A (k^T v) is pair-stacked into
[128,128] matmuls interleaved with the kv projection, accumulating in a
persistent PSUM tile. DMA uses few large kicks ordered by consumption;
output kicks ride the otherwise-idle sync engine; PSUM->SBUF casts are
spread across vector/gpsimd/scalar.
"""

import numpy as np
import ml_dtypes

import concourse.bass as bass
import concourse.mybir as mybir
import concourse.tile as tile
from concourse.bass_utils import run_bass_kernel_spmd
from concourse.vector_clock import ScopedClock
import bass_rust

B, M, H, K, D = 2, 2048, 1024, 16, 64
N_CORES = 8
HPC = 4           # heads per core
CC = HPC * D      # 256 local feature columns per core
P = 128

MM_DT = "f16"


# --- workaround: this walrus rejects multi-wait Drain instructions, so split
# --- the TileContext exit drain into one single-wait drain per proc.
def _split_drain_and_barrier(self, tick_clock, wait_clock):
    n_procs = len(list(tick_clock.global_clock))
    for p, t in enumerate(tick_clock.global_clock):
        if t <= 0:
            continue
        single = bass_rust.VectorClock(
            [t if i == p else 0 for i in range(n_procs)]
        )
        d = self.nc.sync.drain()
        wait_clock.add_sem_waits(d.ins, ScopedClock({None: single}))
    self.nc.all_engine_barrier()
    popped = self.nc._tile_sem_poison_stack.pop()
    assert popped is self._sem_poison
    self.nc.clear_and_free_semaphores(list(self.sems.allocated().values()))
    self.nc.all_engine_barrier()


# --- workaround: the same walrus caps sync waits at 1 per instruction
# --- (2 for EventSemaphore). Tile's wait-assignment can attach more; hoist
# --- the extras onto single-wait nop carriers emitted just before.
_ORIG_COMMIT_AND_LOWER = tile.TileContext._commit_and_lower


def _wait_split_commit_and_lower(self, inst, original_block, old_bb_map,
                                 bb_to_exit_bb):
    si = inst.sync_info
    cap = 2 if isinstance(inst, mybir.InstEventSemaphore) else 1
    ow = list(si.on_wait) if si is not None and si.on_wait else []
    if len(ow) > cap and inst.is_executable():
        for w in ow[:-cap]:
            carrier = self.nc.engines[inst.engine].nop(nofuse=True)
            carrier.ins.sync_info = bass_rust.SyncInfo(
                on_wait=[w], on_update=[]
            )
        inst.sync_info = bass_rust.SyncInfo(
            on_wait=ow[-cap:], on_update=list(si.on_update or [])
        )
    return _ORIG_COMMIT_AND_LOWER(
        self, inst, original_block, old_bb_map, bb_to_exit_bb
    )


if not getattr(tile.TileContext, "_split_drain_patched", False):
    tile.TileContext._drain_and_barrier = _split_drain_and_barrier
    tile.TileContext._commit_and_lower = _wait_split_commit_and_lower
    tile.TileContext._split_drain_patched = True


def _build_nc():
    if MM_DT == "bf16":
        io_dt = mybir.dt.bfloat16
    elif MM_DT == "f16":
        io_dt = mybir.dt.float16
    elif MM_DT == "f32r":
        io_dt = mybir.dt.float32r
    else:
        io_dt = mybir.dt.float32
    f32 = mybir.dt.float32

    nc = bass.Bass()
    IT = H // P           # 8 contraction tiles over feature dim
    LT = M // P           # 16 tiles over sequence dim
    MC = M // 512         # 4 moving chunks over sequence dim
    DT = CC // P          # 2 partition tiles over local feature cols
    JC = H // 512         # 2 chunks over output feature dim
    NCH = 8               # hc DMA chunks (2 seq-tiles each)

    # host pre-arranges every input into its SBUF layout so each DMA kick
    # is one long descriptor per partition (per-ring DMA bandwidth scales
    # with descriptor size). hc is split: four small 128-col chunks for a
    # fast pipeline start, then three 512-col chunks with 8KB descriptors.
    hC = nc.dram_tensor("hC", [P, MC, IT, 512], io_dt, kind="ExternalInput")
    hcA = nc.dram_tensor("hcA", [P, 8, IT, 128], io_dt, kind="ExternalInput")
    hcB = nc.dram_tensor("hcB", [P, 2, IT, 512], io_dt, kind="ExternalInput")
    wq_d = nc.dram_tensor("wq", [P, IT, CC], io_dt, kind="ExternalInput")
    wkv_d = nc.dram_tensor("wkv", [P, IT, 2 * CC], io_dt,
                           kind="ExternalInput")
    wo_d = nc.dram_tensor("wo", [D, HPC, H], io_dt, kind="ExternalInput")
    out_dt = mybir.dt.float16 if MM_DT == "f16" else f32
    outp = nc.dram_tensor("out", [M, H], out_dt, kind="ExternalOutput")

    with tile.TileContext(nc) as tc:
        with (
            tc.tile_pool(name="wp", bufs=1) as wp,
            tc.tile_pool(name="big", bufs=1) as big,
            tc.tile_pool(name="op", bufs=4) as op,
            tc.tile_pool(name="ps", bufs=2, space="PSUM") as ps,
            tc.tile_pool(name="po", bufs=4, space="PSUM") as po,
            tc.tile_pool(name="pa", bufs=2, space="PSUM") as pa,
        ):
            wkv_sb = wp.tile([P, IT, 2 * CC], io_dt, tag="wkv")
            wq_sb = wp.tile([P, IT, CC], io_dt, tag="wq")
            wo_sb = wp.tile([D, HPC, H], io_dt, tag="wo")
            hcA_sb = wp.tile([P, 8, IT, 128], io_dt, tag="hca")
            hcB_sb = wp.tile([P, 2, IT, 512], io_dt, tag="hcb")
            h_sb = wp.tile([P, MC, IT, 512], io_dt, tag="h")

            # per-ring DMA bandwidth is well below the HBM cap, so the kick
            # schedule balances each ring's cumulative bytes against the
            # consumption time of each chunk (P1 eats hc chunks in order)
            nc.sync.dma_start(out=wkv_sb[:, 0:4, :], in_=wkv_d[:, 0:4, :])
            nc.gpsimd.dma_start(out=wkv_sb[:, 4:8, :], in_=wkv_d[:, 4:8, :])
            nc.scalar.dma_start(out=hcA_sb[:, 0], in_=hcA[:, 0])
            nc.gpsimd.dma_start(out=hcA_sb[:, 1], in_=hcA[:, 1])
            nc.scalar.dma_start(out=hcA_sb[:, 2], in_=hcA[:, 2])
            nc.sync.dma_start(out=hcA_sb[:, 3], in_=hcA[:, 3])
            nc.gpsimd.dma_start(out=hcA_sb[:, 4], in_=hcA[:, 4])
            nc.scalar.dma_start(out=hcA_sb[:, 5], in_=hcA[:, 5])
            nc.sync.dma_start(out=hcA_sb[:, 6], in_=hcA[:, 6])
            nc.gpsimd.dma_start(out=hcA_sb[:, 7], in_=hcA[:, 7])
            nc.scalar.dma_start(out=hcB_sb[:, 0], in_=hcB[:, 0])
            nc.gpsimd.dma_start(out=hcB_sb[:, 1], in_=hcB[:, 1])
            nc.sync.dma_start(out=wq_sb[:], in_=wq_d[:])
            nc.sync.dma_start(out=wo_sb[:], in_=wo_d[:])
            engs = [nc.scalar, nc.gpsimd]
            for mc in range(MC):
                engs[mc % 2].dma_start(out=h_sb[:, mc], in_=hC[:, mc])

            def hc_tile(it, lt):
                if lt < 8:
                    return hcA_sb[:, lt, it, :]
                ch, sub = (lt - 8) // 4, (lt - 8) % 4
                return hcB_sb[:, ch, it, sub * P:(sub + 1) * P]

            # persistent intermediates
            kv_sb = big.tile([P, LT, 2 * CC], io_dt, tag="kv")
            q_sb = big.tile([P, DT, M], io_dt, tag="q")
            at_sb = big.tile([D, HPC, D], io_dt, tag="at")
            c_sb = big.tile([P, DT, H], io_dt, tag="c")

            # --- P1 (kv projection) with stage A (pair-stacked v^T k)
            # interleaved one seq-tile behind (so the PE never waits on the
            # kv cast); A accumulates in persistent PSUM tiles, one full
            # bank per head-pair (start=True clears the whole bank).
            at_ps = [
                pa.tile([P, 512], f32, tag="pa", name=f"at_ps{pp}")
                for pp in range(DT)
            ]

            def a_mms(lt):
                for pp in range(DT):
                    nc.tensor.matmul(
                        at_ps[pp][:, 0:P],
                        kv_sb[:, lt, CC + pp * P:CC + (pp + 1) * P],
                        kv_sb[:, lt, pp * P:(pp + 1) * P],
                        start=(lt == 0), stop=(lt == LT - 1),
                        skip_group_check=True,
                    )

            for lt in range(LT):
                acc = ps.tile([P, 2 * CC], f32, tag="ps")
                for it in range(IT):
                    nc.tensor.matmul(
                        acc[:],
                        hc_tile(it, lt),
                        wkv_sb[:, it, :],
                        start=(it == 0), stop=(it == IT - 1),
                    )
                nc.vector.tensor_copy(kv_sb[:, lt, :], acc[:])
                if lt > 0:
                    a_mms(lt - 1)
            a_mms(LT - 1)
            # extract per-head diagonal blocks into baseline at layout
            for hh in range(HPC):
                pp, i = hh // 2, hh % 2
                nc.vector.tensor_copy(
                    at_sb[:, hh, :],
                    at_ps[pp][i * D:(i + 1) * D, i * D:(i + 1) * D],
                )

            def c_stage():
                # rows of (A_h Wo_h^T) [du, j]; uses the po pool (idle
                # until O) and scalar casts (vector is busy with q casts)
                for hh in range(HPC):
                    prow = (hh % 2) * D
                    for jc in range(JC):
                        cacc = po.tile([D, 512], f32, tag="po",
                                       name=f"cacc{hh}_{jc}")
                        nc.tensor.matmul(
                            cacc[:],
                            at_sb[:, hh, :],
                            wo_sb[:, hh, jc * 512:(jc + 1) * 512],
                            start=True, stop=True,
                        )
                        nc.scalar.copy(
                            c_sb[prow:prow + D, hh // 2,
                                 jc * 512:(jc + 1) * 512],
                            cacc[:],
                        )

            # --- P2 (q projection) + O (q @ C) interleaved per 512-col
            # chunk of m; casts rotate across vector/gpsimd/scalar and
            # output kicks ride the sync engine.
            # gpsimd cannot access PSUM; rotate PSUM->SBUF casts over
            # vector and scalar only
            cast_engs = [
                lambda o, i: nc.vector.tensor_copy(o, i),
                lambda o, i: nc.scalar.copy(o, i),
            ]
            n_cast = 0
            def p2_stage(mc):
                for dt_i in range(DT):
                    qacc = ps.tile([P, 512], f32, tag="ps")
                    for it in range(IT):
                        nc.tensor.matmul(
                            qacc[:],
                            wq_sb[:, it, dt_i * P:(dt_i + 1) * P],
                            h_sb[:, mc, it, :],
                            start=(it == 0), stop=(it == IT - 1),
                        )
                    nc.vector.tensor_copy(
                        q_sb[:, dt_i, mc * 512:(mc + 1) * 512], qacc[:]
                    )

            # P2(0) runs between A and C so C's dependency chain (A stop ->
            # extraction cast -> C matmul) resolves off the critical path
            p2_stage(0)
            c_stage()
            for mc in range(MC):
                if mc > 0:
                    p2_stage(mc)
                for mt in range(mc * 4, (mc + 1) * 4):
                    o_t = op.tile([P, H], out_dt, tag="o")
                    # dt-outer: the first two matmuls need only the dt=0
                    # q cast, hiding the dt=1 cast latency
                    oaccs = [po.tile([P, 512], f32, tag="po",
                                     name=f"oacc{mt}_{jc}")
                             for jc in range(JC)]
                    for dt_i in range(DT):
                        for jc in range(JC):
                            nc.tensor.matmul(
                                oaccs[jc][:],
                                q_sb[:, dt_i, mt * P:(mt + 1) * P],
                                c_sb[:, dt_i, jc * 512:(jc + 1) * 512],
                                start=(dt_i == 0), stop=(dt_i == DT - 1),
                                skip_group_check=True,
                            )
                    for jc in range(JC):
                        cast_engs[n_cast % 2](
                            o_t[:, jc * 512:(jc + 1) * 512], oaccs[jc][:]
                        )
                        n_cast += 1
                    # output rides two rings (three for the final tiles so
                    # the last transfers drain fast): per-ring DMA
                    # throughput is well under the HBM cap
                    if mt < 12:
                        oeng = nc.sync if mt % 2 == 0 else nc.gpsimd
                    else:
                        oeng = [nc.sync, nc.gpsimd, nc.scalar,
                                nc.sync][mt - 12]
                    oeng.dma_start(
                        out=outp[mt * P:(mt + 1) * P, :], in_=o_t[:]
                    )

    return nc


_NC_CACHE = {}


def _get_nc():
    if "nc" not in _NC_CACHE:
        _NC_CACHE["nc"] = _build_nc()
    return _NC_CACHE["nc"]


def _cast(a):
    a = np.ascontiguousarray(a)
    if MM_DT == "bf16":
        return a.astype(ml_dtypes.bfloat16)
    if MM_DT == "f16":
        return a.astype(np.float16)
    return a.astype(np.float32)


def make_in_maps(h, h_cache, Wq, Wk, Wv, Wo):
    # pre-arrange every input into its SBUF layout (partition-major with
    # contiguous per-partition chunks) so device DMA descriptors are long
    IT, MC, NCH = H // 128, 4, 8

    def t_chunks(xT, c0, c1, w):
        # [H, M] cols [c0:c1] -> [128 p, nch, IT it, w]
        nch = (c1 - c0) // w
        return _cast(
            np.ascontiguousarray(
                xT[:, c0:c1].reshape(IT, 128, nch, w).transpose(1, 2, 0, 3))
        )

    in_maps = []
    for c in range(N_CORES):
        b, g = divmod(c, 4)
        cols = slice(g * CC, (g + 1) * CC)
        wkvT = np.concatenate([Wk[cols, :].T, Wv[cols, :].T], axis=1)
        in_maps.append({
            "hC": t_chunks(h[b].T, 0, M, 512),
            "hcA": t_chunks(h_cache[b].T, 0, 1024, 128),
            "hcB": t_chunks(h_cache[b].T, 1024, M, 512),
            "wq": _cast(np.ascontiguousarray(
                Wq[cols, :].T.reshape(IT, 128, CC).transpose(1, 0, 2))),
            "wkv": _cast(np.ascontiguousarray(
                wkvT.reshape(IT, 128, 2 * CC).transpose(1, 0, 2))),
            "wo": _cast(np.ascontiguousarray(
                Wo[:, cols].T.reshape(HPC, D, H).transpose(1, 0, 2))),
        })
    return in_maps


def kernel(h, h_cache, key_pe, Wq, Wk, Wv, Wo, _bass_results=None):
    h = np.asarray(h)
    h_cache = np.asarray(h_cache)
    Wq, Wk, Wv, Wo = (np.asarray(a) for a in (Wq, Wk, Wv, Wo))
    nc = _get_nc()
    in_maps = make_in_maps(h, h_cache, Wq, Wk, Wv, Wo)
    res = run_bass_kernel_spmd(nc, in_maps, list(range(N_CORES)))
    if _bass_results is not None:
        _bass_results.append(res)
    out = np.zeros((B, M, H), np.float32)
    for c in range(N_CORES):
        out[c // 4] += res.results[c]["out"].astype(np.float32)
    return out
